# revision 1
# baseline (speedup 1.0000x reference)
"""Trainium2 Bass kernel for nn_DistanceLoss (pairwise SmoothL1 distance loss).

reference:
    t[i,j] = sum_d smoothl1(x[i,d] - x[j,d])   (beta=1)  for x in {teacher, student}
    loss = sum |t/mean(t) - s/mean(s)|

identity used on device (per pair, with d = x_i - x_j):
    smoothl1(d) = 0.5 d^2 - 0.5 relu(|d|-1)^2
    sum_d 0.5 d^2 = 0.5 n_i + 0.5 n_j - G_ij       (Gram decomposition)

Sharding: core k computes rows j in [64k, 64k+64) of the [N,N] pair-sum
matrices for both tensors (layout transposed: d on partitions, i on free dim).
All four terms accumulate into one PSUM tile [64, 512] per tensor via matmuls:
  0.5 n_i : stationary = 0.5-const       [128,64], moving = x^2 tile  [128,512]
  0.5 n_j : stationary = 0.5*xj^2 slice  [128,64], moving = ones      [128,512]
  -G_ij   : stationary = -xj slice       [128,64], moving = x tile    [128,512]
  -0.5 c2 : stationary = -0.5*indicator  [128,64], moving = c2 tile   [128,512]
where c2 = relu(|x_i - x_j| - 1)^2 comes from a fused custom DVE op (some j's
use the Scalar engine's Abs with per-partition bias instead, to balance load).
Host does the final (cheap) mean-normalize + abs-diff reduction in float64.
"""

import sys

for _p in ("/opt/trn_rl_repo", "/root/.axon_site/_ro/trn_rl_repo"):
    if _p not in sys.path:
        sys.path.insert(0, _p)

import numpy as np
import ml_dtypes

N = 512
D = 512
NCORES = 8
JB = N // NCORES  # 64 rows of the pair matrix per core
NT = D // 128  # 4 partition tiles of the transposed layout

# number of j's (out of JB) whose |d| pass runs on the Scalar engine
K_ACT = 44

_CACHE = {}


def _register_custom_op():
    import concourse.dve_ops as dve_ops
    from concourse.dve_spec import Spec, Src0, C0, C1, Zero, maxx, sq, lower
    from concourse.dve_uop import DveOpSpec

    name = "SL1C_ANT"
    for op in dve_ops.OPS:
        if op.name == name:
            return op
    spec = Spec(
        body=sq(maxx(maxx(Src0 - C0, C1 - Src0), Zero)),
        reference=lambda in0, in1, s0, s1, imm2: np.square(
            np.maximum(np.maximum(in0 - s0, s1 - in0), 0.0)
        ).astype(np.float32),
    )
    row = dve_ops._CUSTOM_DVE_ROW_BASE + len(dve_ops.OPS)
    shas = {}
    for ver in ("v3", "v4"):
        s = DveOpSpec(name=name, opcode=row, uops=lower(spec, ver=ver), rd1_en=False)
        shas[ver] = s.sha(ver)
    op = dve_ops.DveOp(name, spec, subdim=False, uops_sha=shas)
    dve_ops.OPS.append(op)
    dve_ops._SUB_OPCODE_FOR_NAME[name] = row
    dve_ops.CUSTOM_DVE_SPECS[name] = spec
    return op


def _build_nc():
    import concourse.bacc as bacc
    import concourse.tile as tile
    from concourse import mybir

    sl1c = _register_custom_op()

    dt = mybir.dt
    nc = bacc.Bacc("TRN2", target_bir_lowering=False, debug=False,
                   num_devices=NCORES)

    dram = {}
    for pfx in ("t", "s"):
        dram[pfx + "_xt"] = nc.dram_tensor(pfx + "_xt", [D, N], dt.bfloat16,
                                           kind="ExternalInput").ap()
        dram[pfx + "_xj"] = nc.dram_tensor(pfx + "_xj", [D, JB], dt.bfloat16,
                                           kind="ExternalInput").ap()
        dram[pfx + "_jp1"] = nc.dram_tensor(pfx + "_jp1", [D, JB], dt.float32,
                                            kind="ExternalInput").ap()
        dram[pfx + "_jm1"] = nc.dram_tensor(pfx + "_jm1", [D, JB], dt.float32,
                                            kind="ExternalInput").ap()
        dram[pfx + "_out"] = nc.dram_tensor(pfx + "_out", [JB, N], dt.float32,
                                            kind="ExternalOutput").ap()

    # evenly spread the ACT-offloaded j's through the loop
    act_js = set()
    if K_ACT > 0:
        step = JB / K_ACT
        act_js = {min(JB - 1, int(i * step)) for i in range(K_ACT)}

    with tile.TileContext(nc) as tc:
        import contextlib

        with contextlib.ExitStack() as ctx:
            singles = ctx.enter_context(tc.tile_pool(name="singles", bufs=1))
            qpool = ctx.enter_context(tc.tile_pool(name="qpool", bufs=4))
            apool = ctx.enter_context(tc.tile_pool(name="apool", bufs=3))
            vpool = ctx.enter_context(tc.tile_pool(name="vpool", bufs=3))
            opool = ctx.enter_context(tc.tile_pool(name="opool", bufs=2))
            psp = ctx.enter_context(tc.tile_pool(name="psp", bufs=2, space="PSUM"))

            # shared constants
            zo = singles.tile([128, 128], dt.bfloat16)  # sliding -0.5 indicator
            nc.vector.memset(zo, 0.0)
            nc.vector.memset(zo[:, 63:64], -0.5)
            half32 = singles.tile([128, JB], dt.float32)
            nc.vector.memset(half32, 0.5)
            ones32 = singles.tile([128, N], dt.float32)
            nc.vector.memset(ones32, 1.0)

            for pfx in ("t", "s"):
                xt_sb = []
                xj_sb = []
                jp1 = []
                jm1 = []
                for t in range(NT):
                    x = singles.tile([128, N], dt.bfloat16, tag=f"{pfx}_xt{t}")
                    nc.sync.dma_start(out=x, in_=dram[pfx + "_xt"][128 * t:128 * (t + 1), :])
                    xt_sb.append(x)
                    xj = singles.tile([128, JB], dt.bfloat16, tag=f"{pfx}_xj{t}")
                    nc.sync.dma_start(out=xj, in_=dram[pfx + "_xj"][128 * t:128 * (t + 1), :])
                    xj_sb.append(xj)
                    p1 = singles.tile([128, JB], dt.float32, tag=f"{pfx}_jp1{t}")
                    nc.sync.dma_start(out=p1, in_=dram[pfx + "_jp1"][128 * t:128 * (t + 1), :])
                    jp1.append(p1)
                    m1 = singles.tile([128, JB], dt.float32, tag=f"{pfx}_jm1{t}")
                    nc.sync.dma_start(out=m1, in_=dram[pfx + "_jm1"][128 * t:128 * (t + 1), :])
                    jm1.append(m1)

                # derived per-tensor tiles
                negxj = []    # bf16, stationary for -G
                negxj32 = []  # fp32, ACT bias (= -xj)
                sq32 = []     # fp32 x^2 tiles, moving for n_i
                hsq32 = []    # fp32 0.5*xj^2 slices, stationary for n_j
                for t in range(NT):
                    nb = singles.tile([128, JB], dt.bfloat16, tag=f"{pfx}_negxj{t}")
                    nc.vector.tensor_scalar(nb, xj_sb[t], -1.0, None, mybir.AluOpType.mult)
                    negxj.append(nb)
                    n32 = singles.tile([128, JB], dt.float32, tag=f"{pfx}_negxj32{t}")
                    # jp1 = xj + 1 (fp32 of the bf16-rounded xj) -> -(jp1 - 1) = -xj
                    nc.vector.tensor_scalar(n32, jp1[t], 1.0, -1.0,
                                            mybir.AluOpType.subtract, mybir.AluOpType.mult)
                    negxj32.append(n32)
                    s32 = singles.tile([128, N], dt.float32, tag=f"{pfx}_sq{t}")
                    nc.vector.tensor_tensor(s32, xt_sb[t], xt_sb[t], mybir.AluOpType.mult)
                    sq32.append(s32)
                    h32 = singles.tile([128, JB], dt.float32, tag=f"{pfx}_hsq{t}")
                    nc.vector.tensor_tensor(h32, xj_sb[t], xj_sb[t], mybir.AluOpType.mult)
                    nc.vector.tensor_scalar(h32, h32, 0.5, None, mybir.AluOpType.mult)
                    hsq32.append(h32)

                acc = psp.tile([JB, N], dt.float32, tag=f"{pfx}_acc")

                # n_i, n_j, -G assembly matmuls
                first = True
                for t in range(NT):
                    nc.tensor.matmul(acc, half32, sq32[t], start=first, stop=False)
                    first = False
                for t in range(NT):
                    nc.tensor.matmul(acc, hsq32[t], ones32, start=False, stop=False)
                for t in range(NT):
                    nc.tensor.matmul(acc, negxj[t], xt_sb[t], start=False, stop=False)

                # per-j correction: c2 = relu(|x_i - x_j| - 1)^2, then
                # matmul with the -0.5 indicator column into row j of acc
                for jl in range(JB):
                    if jl in act_js:
                        a4 = apool.tile([128, NT, N], dt.bfloat16, tag="a4")
                        for t in range(NT):
                            nc.scalar.activation(a4[:, t, :], xt_sb[t],
                                                 mybir.ActivationFunctionType.Abs,
                                                 bias=negxj32[t][:, jl:jl + 1],
                                                 scale=1.0)
                        v4 = vpool.tile([128, NT * N], dt.bfloat16, tag="v4")
                        nc.vector.tensor_scalar(v4, a4.rearrange("p a b -> p (a b)"),
                                                1.0, 0.0, mybir.AluOpType.subtract,
                                                mybir.AluOpType.max)
                        q4 = qpool.tile([128, NT, N], dt.bfloat16, tag="q4")
                        nc.vector.tensor_tensor(q4.rearrange("p a b -> p (a b)"), v4, v4,
                                                mybir.AluOpType.mult)
                    else:
                        q4 = qpool.tile([128, NT, N], dt.bfloat16, tag="q4")
                        for t in range(NT):
                            nc.vector._custom_dve(sl1c, out=q4[:, t, :], in0=xt_sb[t],
                                                  s0=jp1[t][:, jl:jl + 1],
                                                  s1=jm1[t][:, jl:jl + 1])
                    last_j = jl == JB - 1
                    for t in range(NT):
                        nc.tensor.matmul(acc, zo[:, 63 - jl:127 - jl], q4[:, t, :],
                                         start=False,
                                         stop=(last_j and t == NT - 1))

                out_sb = opool.tile([JB, N], dt.float32, tag="out")
                nc.vector.tensor_copy(out_sb, acc)
                nc.sync.dma_start(out=dram[pfx + "_out"], in_=out_sb)

    nc.finalize()
    return nc


def _get_nc():
    if "nc" not in _CACHE:
        _CACHE["nc"] = _build_nc()
    return _CACHE["nc"]


def _prep_inputs(teacher, student):
    in_maps = []
    prepped = {}
    for pfx, x in (("t", teacher), ("s", student)):
        xt32 = np.ascontiguousarray(x.T.astype(np.float32))        # [D, N]
        xtb = xt32.astype(ml_dtypes.bfloat16)                       # [D, N] bf16
        xtb32 = xtb.astype(np.float32)  # bf16-rounded values, exact in fp32
        prepped[pfx] = (xtb, xtb32)
    for k in range(NCORES):
        sl = slice(JB * k, JB * (k + 1))
        m = {}
        for pfx in ("t", "s"):
            xtb, xtb32 = prepped[pfx]
            m[pfx + "_xt"] = xtb
            m[pfx + "_xj"] = np.ascontiguousarray(xtb[:, sl])
            m[pfx + "_jp1"] = np.ascontiguousarray(xtb32[:, sl] + 1.0)
            m[pfx + "_jm1"] = np.ascontiguousarray(xtb32[:, sl] - 1.0)
        in_maps.append(m)
    return in_maps


def run_device(teacher, student, **kwargs):
    """Run the device part; returns (T, S) pair-sum matrices [N, N] (j, i)."""
    from concourse.bass_utils import run_bass_kernel_spmd

    nc = _get_nc()
    in_maps = _prep_inputs(teacher, student)
    res = run_bass_kernel_spmd(nc, in_maps, core_ids=list(range(NCORES)), **kwargs)
    T = np.concatenate([res.results[k]["t_out"] for k in range(NCORES)], axis=0)
    S = np.concatenate([res.results[k]["s_out"] for k in range(NCORES)], axis=0)
    return T, S, res


def kernel(teacher, student):
    teacher = np.asarray(teacher)
    student = np.asarray(student)
    T, S, _ = run_device(teacher, student)
    T64 = T.astype(np.float64)
    S64 = S.astype(np.float64)
    out = np.abs(T64 / T64.mean() - S64 / S64.mean()).sum()
    return np.float32(out)


if __name__ == "__main__":
    rng = np.random.default_rng(0)
    t = rng.standard_normal((N, D)).astype(np.float32)
    s = rng.standard_normal((N, D)).astype(np.float32)
    print(kernel(t, s))
